# revision 7
# baseline (speedup 1.0000x reference)
"""Trainium2 Bass kernel: dark-channel + 15x15 erosion (min-pool, stride 1,
+inf padding), data-parallel over 8 NeuronCores.

Input  I: [32, 3, 512, 512] f32, k: scalar (15)
Output:   [32, 1, 512, 512] f32  (min over channels, then kxk spatial min)

Per-core plan (4 images each), pipelined over half-images:
  1. DMA half-image (one tile per channel) into SBUF, rows on partitions.
  2. Channel min on GpSimd (2 tensor_tensor min ops) -> padded f16 row buffer.
  3. Horizontal 15-min-filter on DVE: dyadic shifted mins (1,2,4,7).
  4. PE transpose (identity matmul) + ScalarE PSUM evac -> column layout.
  5. Vertical 15-min-filter on DVE (same dyadic trick along free dim).
  6. PE transpose back + ScalarE evac (f16 -> f32 cast) -> row layout.
  7. DMA result to HBM.

fp16 intermediates: values are mins of uniform[0,1) data; min is selection,
not arithmetic, so fp16 keeps rel err ~1e-4. Pad value 30000.0 acts as +inf.
Padded buffers are persistent ping-pong tiles so pad regions are set once.

INTERLEAVE mode packs two independent streams (row-tile pairs for the
h-pass, column-block pairs for the v-pass) element-interleaved along the
free dim, so every DVE shift is an even element count = 4-byte aligned,
keeping all fp16 tensor_tensor ops in the 2x_1P perf mode on hardware.
De-interleave is free: GpSimd/ScalarE write strided (4B stride), the PE
transpose reads strided.
"""

import sys

if "/opt/trn_rl_repo" not in sys.path:
    sys.path.insert(0, "/opt/trn_rl_repo")

import numpy as np

N_CORES = 8
IMGS = 4          # images per core
C = 3
H = W = 512
K = 15
PAD = K // 2      # 7
L = 8             # left pad in filter buffers (>= PAD+1, power of 2)
PITCH = L + 512 + 8   # 528, padded row/col length (logical)
NJ = H // 128     # row tiles
NB = W // 128     # col blocks
JH = NJ // 2      # row tiles per half-image
PADV = 30000.0    # effective +inf for data in [0,1)

_cache = {}


def _build_nc(use_f16=True, interleave=True, io_bufs=4, scr_bufs=3,
              fx_bufs=4, res_bufs=6, out_bufs=2, psum_bufs=8):
    import concourse.bass as bass
    import concourse.mybir as mybir
    import concourse.tile as tile
    import concourse.masks as masks

    F32 = mybir.dt.float32
    FI = mybir.dt.float16 if use_f16 else F32
    MIN = mybir.AluOpType.min
    S = 2 if interleave else 1      # physical stride of one logical stream

    nc = bass.Bass("TRN2", target_bir_lowering=False, debug=False)
    inp = nc.dram_tensor("inp", [IMGS, C, H, W], F32, kind="ExternalInput")
    out = nc.dram_tensor("out", [IMGS, 1, H, W], F32, kind="ExternalOutput")

    def dyadic(pool, src, n):
        """15-wide min filter along the last dim of src [128, n, S*PITCH].
        Logical x sits at [S*L : S*(L+512)]; shifts scale by S (so with
        interleave every operand offset is even = 4B aligned -> 2x mode).
        Returns tile [128, n, S*512]: res[S*i+par] = min over the 15-window
        of stream par at logical position i."""
        P = S * PITCH
        f2 = pool.tile([128, n, P], FI, tag="fa")
        nc.vector.tensor_tensor(
            f2[:, :, 0 : S * 526], src[:, :, 0 : S * 526],
            src[:, :, S * 1 : S * 527], op=MIN,
        )
        f4 = pool.tile([128, n, P], FI, tag="fb")
        nc.vector.tensor_tensor(
            f4[:, :, 0 : S * 524], f2[:, :, 0 : S * 524],
            f2[:, :, S * 2 : S * 526], op=MIN,
        )
        f8 = pool.tile([128, n, P], FI, tag="fa")
        nc.vector.tensor_tensor(
            f8[:, :, 0 : S * 520], f4[:, :, 0 : S * 520],
            f4[:, :, S * 4 : S * 524], op=MIN,
        )
        res = pool.tile([128, n, S * 512], FI, tag="res")
        nc.vector.tensor_tensor(
            res[:], f8[:, :, S * 1 : S * 513], f8[:, :, S * 8 : S * 520],
            op=MIN,
        )
        return res

    with tile.TileContext(nc) as tc:
        with (
            tc.tile_pool(name="const", bufs=1) as cpool,
            tc.tile_pool(name="io", bufs=io_bufs) as io_pool,
            tc.tile_pool(name="scrp", bufs=scr_bufs) as scrp,
            tc.tile_pool(name="work", bufs=fx_bufs) as work,
            tc.tile_pool(name="resp", bufs=res_bufs) as resp,
            tc.tile_pool(name="opool", bufs=out_bufs) as opool,
            tc.tile_pool(name="psum", bufs=psum_bufs, space="PSUM") as psum,
        ):
            ident = cpool.tile([128, 128], FI)
            masks.make_identity(nc, ident[:])

            # persistent padded buffers (ping-pong across images); pad
            # columns are written once here and never touched again.
            # layout: xpad [128, NJ//S, S*PITCH]  (h-pass, row-tile streams)
            #         vb   [128, NB//S, S*PITCH]  (v-pass, col-block streams)
            xpads, vbs = [], []
            for pp in range(2):
                xp = cpool.tile([128, NJ // S, S * PITCH], FI, tag=f"xpad{pp}")
                nc.gpsimd.memset(xp[:, :, 0 : S * L], PADV)
                nc.gpsimd.memset(xp[:, :, S * (L + W) : S * PITCH], PADV)
                xpads.append(xp)
                vb = cpool.tile([128, NB // S, S * PITCH], FI, tag=f"vb{pp}")
                nc.gpsimd.memset(vb[:, :, 0 : S * L], PADV)
                nc.gpsimd.memset(vb[:, :, S * (L + H) : S * PITCH], PADV)
                vbs.append(vb)

            for i in range(IMGS):
                xpad = xpads[i % 2]
                vb = vbs[i % 2]

                # --- per half-image: load + channel-min + h-filter
                r_halves = []
                for hh in range(2):
                    # one tile per channel => each consumer waits on <= 2
                    # DMA semaphores (HW limit on sync-wait commands).
                    tcs = []
                    for c in range(C):
                        t = io_pool.tile([128, JH, W], F32, tag=f"tc{c}")
                        nc.sync.dma_start(
                            t[:],
                            inp[i, c, 256 * hh : 256 * (hh + 1)].rearrange(
                                "(j p) w -> p j w", p=128
                            ),
                        )
                        tcs.append(t)
                    scr = scrp.tile([128, JH, W], F32)
                    nc.gpsimd.tensor_tensor(
                        scr[:], tcs[0][:], tcs[1][:], op=MIN
                    )
                    if interleave:
                        # write the two row-tiles of this half interleaved:
                        # phys = 2*(L+w) + j  (stride-2 writes, 4B stride)
                        xdst = xpad[:, hh, :].rearrange(
                            "p (w s) -> p s w", s=2
                        )[:, :, L : L + W]
                        # xdst dims: [p, j(2, step 1), w(512, step 2)]
                        nc.gpsimd.tensor_tensor(
                            xdst, scr[:], tcs[2][:], op=MIN
                        )
                        r_halves.append(dyadic(work, xpad[:, hh : hh + 1, :], 1))
                    else:
                        xslice = xpad[:, 2 * hh : 2 * (hh + 1), :]
                        nc.gpsimd.tensor_tensor(
                            xslice[:, :, L : L + W], scr[:], tcs[2][:],
                            op=MIN,
                        )
                        r_halves.append(dyadic(work, xslice, JH))

                # --- transpose to column layout
                for j in range(NJ):
                    rh = r_halves[j // JH]
                    for b in range(NB):
                        if interleave:
                            # stream par = j % 2 within the half's pair
                            rsl = rh[:, 0, :].rearrange("p (w s) -> p s w", s=2)[
                                :, j % 2, 128 * b : 128 * (b + 1)
                            ]
                        else:
                            rsl = rh[:, j % JH, 128 * b : 128 * (b + 1)]
                        pt = psum.tile([128, 128], FI)
                        nc.tensor.transpose(pt[:], rsl, ident[:])
                        if interleave:
                            vdst = vb[:, b // 2, :].rearrange(
                                "p (w s) -> p s w", s=2
                            )[:, b % 2, L + 128 * j : L + 128 * (j + 1)]
                        else:
                            vdst = vb[:, b, L + 128 * j : L + 128 * (j + 1)]
                        nc.scalar.copy(vdst, pt[:])

                # --- vertical filter per column-block group
                u_pairs = [
                    dyadic(work, vb[:, bp : bp + 1, :]
                           if interleave else vb[:, 2 * bp : 2 * (bp + 1), :],
                           1 if interleave else 2)
                    for bp in range(2)
                ]

                # --- transpose back, f32 out
                o = opool.tile([128, NJ, W], F32)
                for hh in range(2):
                    for j in range(JH * hh, JH * (hh + 1)):
                        for b in range(NB):
                            up = u_pairs[b // 2]
                            if interleave:
                                usl = up[:, 0, :].rearrange(
                                    "p (h s) -> p s h", s=2
                                )[:, b % 2, 128 * j : 128 * (j + 1)]
                            else:
                                usl = up[:, b % 2, 128 * j : 128 * (j + 1)]
                            pt = psum.tile([128, 128], FI)
                            nc.tensor.transpose(pt[:], usl, ident[:])
                            nc.scalar.copy(
                                o[:, j, 128 * b : 128 * (b + 1)], pt[:]
                            )
                    # --- store half-image
                    nc.sync.dma_start(
                        out[i, 0, 256 * hh : 256 * (hh + 1)].rearrange(
                            "(j p) w -> p j w", p=128
                        ),
                        o[:, JH * hh : JH * (hh + 1), :],
                    )
    return nc


def _get_nc():
    if "nc" not in _cache:
        _cache["nc"] = _build_nc()
    return _cache["nc"]


def kernel(I, k):
    from concourse.bass_utils import run_bass_kernel_spmd

    k = int(np.asarray(k))
    assert k == K, f"kernel compiled for k={K}, got {k}"
    I = np.ascontiguousarray(np.asarray(I), dtype=np.float32)
    B = I.shape[0]
    assert I.shape == (B, C, H, W) and B == N_CORES * IMGS

    nc = _get_nc()
    in_maps = [
        {"inp": I[c * IMGS : (c + 1) * IMGS]} for c in range(N_CORES)
    ]
    res = run_bass_kernel_spmd(nc, in_maps, list(range(N_CORES))).results
    return np.concatenate([res[c]["out"] for c in range(N_CORES)], axis=0)
